# revision 1
# baseline (speedup 1.0000x reference)
"""HGNN metapath GRU + edge-softmax message passing on 8 TRN2 NeuronCores.

Strategy (self-contained, full inputs in / full output out):
 - Edges are sharded by DESTINATION NODE RANGE: core c owns nodes
   [c*2500, (c+1)*2500) and every edge whose dst lands there (host sorts
   edges by dst).  All segment ops (softmax sum + message scatter) are then
   core-local: zero collectives.
 - The final two linear layers are folded through the segment-sum:
   out[n] = sum_h Q[n,h,:]/S[n,h] + bc,  where per-edge
   q[e,(h,i)] = exp(lrelu(a))[e,h] * (eft[e] @ BA)[.,(h,i)] is scattered
   with one-hot matmuls (one-hot matrices precomputed on host from indices).
 - GRU runs feature-major (gate dims on partitions, edges on the free dim);
   node embeddings are gathered feature-major via dma_gather(transpose=True)
   from a bf16 node-major DRAM table computed on-device.
"""

import sys
import numpy as np

sys.path.insert(0, "/opt/trn_rl_repo")

import ml_dtypes  # noqa: E402

N_NODES = 20000
N_CORES = 8
NPC = N_NODES // N_CORES          # 2500 nodes per core
NODE_CHUNKS = (NPC + 127) // 128  # 20
WALK = 4
FEAT = 256
HID = 64
NH = 8
HR = NH * HID                     # 512
G3 = 3 * HR                       # 1536
OUT_DIM = 16
E_TILE = 512
NP_PAD = ((N_NODES + 511) // 512) * 512  # 20480 padded node rows

bf = ml_dtypes.bfloat16


def _wrap_idx(v):
    """int array [n] -> wrapped int16 [128, n//16] layout for dma_gather."""
    n = v.shape[0]
    assert n % 16 == 0
    w = v.reshape(n // 16, 16).T.astype(np.int16)      # [16, n//16]
    return np.tile(w, (8, 1))                           # [128, n//16]


def _host_prep(x, W_mlp, b_mlp, W_ih, W_hh, b_ih, b_hh, attn, W_emb, b_emb,
               W_last, b_last, edge_metapath_indices):
    idx = np.asarray(edge_metapath_indices).astype(np.int64)
    E = idx.shape[0]
    dst = idx[:, -1]
    core = np.clip(dst // NPC, 0, N_CORES - 1)

    per_core_eids = []
    for c in range(N_CORES):
        sel = np.nonzero(core == c)[0]
        order = np.argsort(dst[sel], kind="stable")
        per_core_eids.append(sel[order])
    counts = [len(e) for e in per_core_eids]
    E_pad = max(512, ((max(counts) + E_TILE - 1) // E_TILE) * E_TILE)
    n_tiles = E_pad // E_TILE
    n_ech = E_pad // 128

    # per-core sorted/padded indices + local dst
    sidx = np.zeros((N_CORES, E_pad, WALK), np.int64)
    ldst = np.full((N_CORES, E_pad), -1000, np.int64)
    for c in range(N_CORES):
        e = per_core_eids[c]
        sidx[c, :len(e)] = idx[e]
        ldst[c, :len(e)] = dst[e] - c * NPC

    # gather indices: per tile, 4*E_TILE idxs (step-major)
    gidx = np.zeros((N_CORES, n_tiles, 128, (WALK * E_TILE) // 16), np.int16)
    for c in range(N_CORES):
        for t in range(n_tiles):
            v = sidx[c, t * E_TILE:(t + 1) * E_TILE, :].T.reshape(-1)  # [4*E_TILE]
            gidx[c, t] = _wrap_idx(v)

    # shared scatter schedule: union over cores of node-chunks touched per edge-chunk
    pairs = []            # list of (k, j)
    pair_of = {}
    for k in range(n_ech):
        js = set()
        for c in range(N_CORES):
            d = ldst[c, k * 128:(k + 1) * 128]
            js |= set((d[d >= 0] // 128).tolist())
        if js:
            for j in range(min(js), max(js) + 1):
                pair_of[(k, j)] = len(pairs)
                pairs.append((k, j))
    first_k, last_k = {}, {}
    for (k, j) in pairs:
        first_k.setdefault(j, k)
        last_k[j] = k
    # max concurrently-active accumulators
    active, max_active = set(), 0
    for k in range(n_ech):
        for (kk, j) in pairs:
            if kk == k:
                if first_k[j] == k:
                    active.add(j)
        max_active = max(max_active, len(active))
        for j in list(active):
            if last_k[j] == k:
                active.discard(j)
    n_pairs = len(pairs)

    oneh = np.zeros((N_CORES, max(n_pairs, 1), 128, 128), bf)
    m_ids = np.arange(128)
    for c in range(N_CORES):
        for p, (k, j) in enumerate(pairs):
            d = ldst[c, k * 128:(k + 1) * 128]
            oneh[c, p] = (d[:, None] == (j * 128 + m_ids)[None, :]).astype(bf)

    # weights
    Wc = (np.asarray(W_last, np.float32) @ np.asarray(W_emb, np.float32))  # [16, 512]
    BA = np.zeros((HR, 136), np.float32)
    attn = np.asarray(attn, np.float32)
    for h in range(NH):
        BA[h * HID:(h + 1) * HID, h * OUT_DIM:(h + 1) * OUT_DIM] = \
            Wc[:, h * HID:(h + 1) * HID].T
        BA[h * HID:(h + 1) * HID, 128 + h] = attn[0, h, :]
    ba_p = BA.reshape(4, 128, 136).transpose(1, 0, 2).reshape(128, 4 * 136).astype(bf)

    W_hhT = np.asarray(W_hh, np.float32).T                       # [512, 1536]
    whh_p = W_hhT.reshape(4, 128, G3).transpose(1, 0, 2).reshape(128, 4 * G3).astype(bf)
    wih_p = np.asarray(W_ih, np.float32).T.astype(bf)            # [64, 1536]
    wmlp_p = np.asarray(W_mlp, np.float32).T.astype(bf)          # [256, 64]

    b_ih = np.asarray(b_ih, np.float32)
    b_hh = np.asarray(b_hh, np.float32)
    brz = (b_ih + b_hh)[:2 * HR].reshape(8, 128).T.copy()        # [128, 8]
    bnih = b_ih[2 * HR:].reshape(4, 128).T.copy()                # [128, 4]
    bnhh = b_hh[2 * HR:].reshape(4, 128).T.copy()                # [128, 4]
    has_bnhh = bool(np.any(bnhh != 0.0))

    b_mlp = np.asarray(b_mlp, np.float32)
    has_bmlp = bool(np.any(b_mlp != 0.0))
    bmlp_row = np.tile(b_mlp[None, :], (128, 1)).astype(np.float32)  # [128, 64]

    bc_vec = (np.asarray(b_emb, np.float32) @ np.asarray(W_last, np.float32).T
              + np.asarray(b_last, np.float32))                  # [16]
    bc_t = np.tile(bc_vec[None, :], (128, 1)).astype(np.float32)

    x_pad = np.zeros((NP_PAD, FEAT), np.float32)
    x_pad[:N_NODES] = np.asarray(x, np.float32)

    plan = dict(E_pad=E_pad, n_tiles=n_tiles, n_ech=n_ech, pairs=pairs,
                pair_of=pair_of, first_k=first_k, last_k=last_k,
                max_active=max_active, n_pairs=n_pairs,
                has_bnhh=has_bnhh, has_bmlp=has_bmlp,
                flushed=set(last_k.keys()), bc_vec=bc_vec)
    wmlp_pk = wmlp_p.reshape(2, 128, HID).transpose(1, 0, 2).reshape(128, 2 * HID)
    shared = dict(x=x_pad, wmlp=np.ascontiguousarray(wmlp_pk), wih=wih_p,
                  whh=whh_p, ba=ba_p,
                  brz=brz, bnih=bnih, bnhh=bnhh, bmlp=bmlp_row, bc=bc_t)
    percore = dict(gidx=gidx, oneh=oneh)
    return plan, shared, percore


def _build(plan, phases=3):
    from contextlib import ExitStack
    import concourse.bass as bass
    import concourse.tile as tile
    from concourse import bacc, mybir

    f32 = mybir.dt.float32
    bf16 = mybir.dt.bfloat16
    i16 = mybir.dt.int16
    AF = mybir.ActivationFunctionType
    OP = mybir.AluOpType
    P = 128

    E_pad, n_tiles, n_ech = plan["E_pad"], plan["n_tiles"], plan["n_ech"]
    pairs, pair_of = plan["pairs"], plan["pair_of"]
    first_k, last_k = plan["first_k"], plan["last_k"]
    has_bnhh, has_bmlp = plan["has_bnhh"], plan["has_bmlp"]
    acc_bufs = min(6, max(2, plan["max_active"] + 1))

    nc = bacc.Bacc("TRN2", target_bir_lowering=False, debug=False)

    x_d = nc.dram_tensor("x", [NP_PAD, FEAT], f32, kind="ExternalInput")
    wmlp_d = nc.dram_tensor("wmlp", [P, 2 * HID], bf16, kind="ExternalInput")
    wih_d = nc.dram_tensor("wih", [HID, G3], bf16, kind="ExternalInput")
    whh_d = nc.dram_tensor("whh", [P, 4 * G3], bf16, kind="ExternalInput")
    ba_d = nc.dram_tensor("ba", [P, 4 * 136], bf16, kind="ExternalInput")
    brz_d = nc.dram_tensor("brz", [P, 8], f32, kind="ExternalInput")
    bnih_d = nc.dram_tensor("bnih", [P, 4], f32, kind="ExternalInput")
    bnhh_d = nc.dram_tensor("bnhh", [P, 4], f32, kind="ExternalInput")
    bmlp_d = nc.dram_tensor("bmlp", [P, HID], f32, kind="ExternalInput")
    bc_d = nc.dram_tensor("bc", [P, OUT_DIM], f32, kind="ExternalInput")
    gidx_d = nc.dram_tensor("gidx", [n_tiles, P, (WALK * E_TILE) // 16], i16,
                            kind="ExternalInput")
    oneh_d = nc.dram_tensor("oneh", [max(plan["n_pairs"], 1), P, P], bf16,
                            kind="ExternalInput")
    out_d = nc.dram_tensor("out", [NODE_CHUNKS * P, OUT_DIM], f32,
                           kind="ExternalOutput")
    etab_d = nc.dram_tensor("etab", [NP_PAD, P], bf16, kind="Internal")

    from concourse.masks import make_identity

    with tile.TileContext(nc) as tc, ExitStack() as ctx:
        wpool = ctx.enter_context(tc.tile_pool(name="w", bufs=1))
        wih_sb = wpool.tile([HID, G3], bf16, tag="wih")
        nc.sync.dma_start(wih_sb[:], wih_d[:])
        whh_sb = wpool.tile([P, 4 * G3], bf16, tag="whh")
        nc.sync.dma_start(whh_sb[:], whh_d[:])
        ba_sb = wpool.tile([P, 4 * 136], bf16, tag="ba")
        nc.sync.dma_start(ba_sb[:], ba_d[:])
        brz_sb = wpool.tile([P, 8], f32, tag="brz")
        nc.sync.dma_start(brz_sb[:], brz_d[:])
        bnih_sb = wpool.tile([P, 4], f32, tag="bnih")
        nc.sync.dma_start(bnih_sb[:], bnih_d[:])
        bnhh_sb = wpool.tile([P, 4], f32, tag="bnhh")
        nc.sync.dma_start(bnhh_sb[:], bnhh_d[:])
        bmlp_sb = wpool.tile([P, HID], f32, tag="bmlp")
        nc.sync.dma_start(bmlp_sb[:], bmlp_d[:])
        bc_sb = wpool.tile([P, OUT_DIM], f32, tag="bc")
        nc.sync.dma_start(bc_sb[:], bc_d[:])
        wm_sb = wpool.tile([P, 2 * HID], bf16, tag="wm")  # packed k-chunks of W_mlp.T
        nc.sync.dma_start(wm_sb[:], wmlp_d[:])
        ident = wpool.tile([P, P], f32, tag="ident")
        make_identity(nc, ident[:])

        hTf = None
        if phases in (2, 3):
            hpool = ctx.enter_context(tc.tile_pool(name="hT", bufs=1))
            hTf = [hpool.tile([P, E_pad], bf16, tag=f"h{c}", name=f"hTf{c}")
                   for c in range(4)]

        # ---------------- phase 1: embedding table ----------------
        with tc.tile_pool(name="e_sb", bufs=3) as epool, \
             tc.tile_pool(name="e_ps", bufs=3, space="PSUM") as epsum:
            n_nchunks = NP_PAD // P  # 160
            for cchunk in range(n_nchunks):
                r0 = cchunk * P
                xin = epool.tile([P, FEAT], f32, tag="xin")
                nc.sync.dma_start(xin[:], x_d[r0:r0 + P, :])
                xT = []
                for half in range(2):
                    pt = epsum.tile([P, P], f32, tag="pt", space="PSUM")
                    nc.tensor.transpose(pt[:], xin[:, half * P:(half + 1) * P],
                                        ident[:])
                    xt = epool.tile([P, P], bf16, tag=f"xt{half}")
                    if half == 0:
                        nc.vector.tensor_copy(xt[:], pt[:])
                    else:
                        nc.scalar.copy(xt[:], pt[:])
                    xT.append(xt)
                ep = epsum.tile([P, HID], f32, tag="ep", space="PSUM")
                nc.tensor.matmul(ep[:], xT[0][:], wm_sb[:, 0:HID],
                                 start=True, stop=False)
                nc.tensor.matmul(ep[:], xT[1][:], wm_sb[:, HID:2 * HID],
                                 start=False, stop=True)
                esb = epool.tile([P, P], bf16, tag="esb")
                nc.vector.memset(esb[:, HID:P], 0)
                if has_bmlp:
                    # emb rows: bias varies along free dim; add via replicated tile
                    nc.vector.tensor_tensor(esb[:, 0:HID], ep[:], bmlp_sb[:],
                                            OP.add)
                else:
                    nc.scalar.copy(esb[:, 0:HID], ep[:])
                nc.sync.dma_start(etab_d[r0:r0 + P, :], esb[:])

        # ---------------- phase 2: GRU over edge tiles ----------------
        NIDX = WALK * E_TILE
        if phases < 2:
            n_tiles_run = 0
        else:
            n_tiles_run = n_tiles
        with tc.tile_pool(name="g_idx", bufs=2) as ipool, \
             tc.tile_pool(name="g_gat", bufs=2) as gpool, \
             tc.tile_pool(name="g_rzn", bufs=2) as rznpool, \
             tc.tile_pool(name="g_h", bufs=2) as hspool, \
             tc.tile_pool(name="g_tmp", bufs=2) as tpool, \
             tc.tile_pool(name="g_ps", bufs=6, space="PSUM") as gpsum:

            def wih_s(m):
                return wih_sb[:, m * P:(m + 1) * P]

            def whh_s(k, m):
                return whh_sb[:, k * G3 + m * P:k * G3 + (m + 1) * P]

            for t in range(n_tiles_run):
                idxt = ipool.tile([P, NIDX // 16], i16, tag="idx")
                nc.sync.dma_start(idxt[:], gidx_d[t])
                gat = gpool.tile([P, 1, NIDX], bf16, tag="gat")
                nc.gpsimd.dma_gather(gat[:], etab_d[:], idxt[:], NIDX, NIDX, P,
                                     transpose=True, single_packet=False)
                if phases == 15:  # gather-only debug mode
                    sink = tpool.tile([P, 16], f32, tag="sink")
                    nc.vector.tensor_copy(sink[:], gat[0:P, 0, 0:16])
                    nc.sync.dma_start(out_d[0:P, 0:OUT_DIM], sink[:, 0:OUT_DIM])
                    continue

                def x_s(s):
                    return gat[0:HID, 0, s * E_TILE:(s + 1) * E_TILE]

                h_cur = [None] * 4
                # ---- step 0 (h = 0)
                r0_sb = [None] * 4
                if has_bnhh:
                    for c in range(4):
                        p = gpsum.tile([P, E_TILE], f32, tag="g", space="PSUM")
                        nc.tensor.matmul(p[:], wih_s(c), x_s(0), start=True, stop=True)
                        r0 = rznpool.tile([P, E_TILE], f32, tag=f"rz{c}")
                        nc.scalar.activation(r0[:], p[:], AF.Sigmoid,
                                             bias=brz_sb[:, c:c + 1])
                        r0_sb[c] = r0
                z0_sb = [None] * 4
                for c in range(4):
                    p = gpsum.tile([P, E_TILE], f32, tag="g", space="PSUM")
                    nc.tensor.matmul(p[:], wih_s(4 + c), x_s(0), start=True, stop=True)
                    z0 = rznpool.tile([P, E_TILE], bf16, tag=f"rz{4 + c}")
                    nc.scalar.activation(z0[:], p[:], AF.Sigmoid,
                                         bias=brz_sb[:, 4 + c:5 + c])
                    z0_sb[c] = z0
                for c in range(4):
                    p = gpsum.tile([P, E_TILE], f32, tag="g", space="PSUM")
                    nc.tensor.matmul(p[:], wih_s(8 + c), x_s(0), start=True, stop=True)
                    n0 = rznpool.tile([P, E_TILE], bf16, tag=f"n{c}")
                    if has_bnhh:
                        rb = tpool.tile([P, E_TILE], f32, tag="rb")
                        nc.vector.tensor_scalar(rb[:], r0_sb[c][:],
                                                bnhh_sb[:, c:c + 1], None, OP.mult)
                        npre = tpool.tile([P, E_TILE], bf16, tag="npre")
                        nc.vector.tensor_tensor(npre[:], rb[:], p[:], OP.add)
                        nc.scalar.activation(n0[:], npre[:], AF.Tanh,
                                             bias=bnih_sb[:, c:c + 1])
                    else:
                        nc.scalar.activation(n0[:], p[:], AF.Tanh,
                                             bias=bnih_sb[:, c:c + 1])
                    zn = tpool.tile([P, E_TILE], bf16, tag="zn")
                    nc.vector.tensor_tensor(zn[:], z0_sb[c][:], n0[:], OP.mult)
                    h0 = hspool.tile([P, E_TILE], bf16, tag=f"h{c}")
                    nc.vector.tensor_tensor(h0[:], n0[:], zn[:], OP.subtract)
                    h_cur[c] = h0

                # ---- steps 1..3
                for s in range(1, WALK):
                    final = (s == WALK - 1)
                    rz_sb = []
                    for m in range(8):
                        p = gpsum.tile([P, E_TILE], f32, tag="g", space="PSUM")
                        nc.tensor.matmul(p[:], wih_s(m), x_s(s),
                                         start=True, stop=False)
                        for k in range(4):
                            nc.tensor.matmul(p[:], whh_s(k, m), h_cur[k][:],
                                             start=False, stop=(k == 3))
                        rz = rznpool.tile([P, E_TILE], bf16, tag=f"rz{m}")
                        nc.scalar.activation(rz[:], p[:], AF.Sigmoid,
                                             bias=brz_sb[:, m:m + 1])
                        rz_sb.append(rz)
                    n_sb = []
                    for c in range(4):
                        pxn = gpsum.tile([P, E_TILE], f32, tag="g", space="PSUM")
                        nc.tensor.matmul(pxn[:], wih_s(8 + c), x_s(s),
                                         start=True, stop=True)
                        phn = gpsum.tile([P, E_TILE], f32, tag="g", space="PSUM")
                        for k in range(4):
                            nc.tensor.matmul(phn[:], whh_s(k, 8 + c), h_cur[k][:],
                                             start=(k == 0), stop=(k == 3))
                        rhn = tpool.tile([P, E_TILE], f32, tag="rhn")
                        if has_bnhh:
                            phb = tpool.tile([P, E_TILE], f32, tag="phb")
                            nc.vector.tensor_scalar(phb[:], phn[:],
                                                    bnhh_sb[:, c:c + 1], None, OP.add)
                            nc.vector.tensor_tensor(rhn[:], rz_sb[c][:], phb[:],
                                                    OP.mult)
                        else:
                            nc.vector.tensor_tensor(rhn[:], rz_sb[c][:], phn[:],
                                                    OP.mult)
                        npre = tpool.tile([P, E_TILE], bf16, tag="npre")
                        nc.vector.tensor_tensor(npre[:], rhn[:], pxn[:], OP.add)
                        nn = rznpool.tile([P, E_TILE], bf16, tag=f"n{c}")
                        nc.scalar.activation(nn[:], npre[:], AF.Tanh,
                                             bias=bnih_sb[:, c:c + 1])
                        n_sb.append(nn)
                    for c in range(4):
                        d = tpool.tile([P, E_TILE], bf16, tag="d")
                        nc.vector.tensor_tensor(d[:], h_cur[c][:], n_sb[c][:],
                                                OP.subtract)
                        zd = tpool.tile([P, E_TILE], bf16, tag="zd")
                        nc.vector.tensor_tensor(zd[:], rz_sb[4 + c][:], d[:], OP.mult)
                        if final:
                            hn_ap = hTf[c][:, t * E_TILE:(t + 1) * E_TILE]
                            nc.vector.tensor_tensor(hn_ap, n_sb[c][:], zd[:], OP.add)
                            h_cur[c] = None
                        else:
                            hn = hspool.tile([P, E_TILE], bf16, tag=f"h{c}")
                            nc.vector.tensor_tensor(hn[:], n_sb[c][:], zd[:], OP.add)
                            h_cur[c] = hn

        # ---------------- phase 3: attention + one-hot scatter ----------------
        with tc.tile_pool(name="s_sb", bufs=2) as spool, \
             tc.tile_pool(name="s_oh", bufs=4) as ohpool, \
             tc.tile_pool(name="s_pay", bufs=3) as paypool, \
             tc.tile_pool(name="s_ps", bufs=2, space="PSUM") as papsum, \
             tc.tile_pool(name="s_acc", bufs=acc_bufs, space="PSUM") as accpsum:

            chunk_pairs = {}
            if phases != 3:
                pairs = []
            for (k, j) in pairs:
                chunk_pairs.setdefault(k, []).append(j)
            acc = {}
            for k in range(n_ech):
                js = chunk_pairs.get(k)
                if not js:
                    continue
                pa = papsum.tile([P, 136], f32, tag="pa", space="PSUM")
                for kk in range(4):
                    nc.tensor.matmul(pa[:], hTf[kk][:, k * P:(k + 1) * P],
                                     ba_sb[:, kk * 136:(kk + 1) * 136],
                                     start=(kk == 0), stop=(kk == 3))
                asb = spool.tile([P, NH], f32, tag="asb")
                nc.vector.tensor_scalar(asb[:], pa[:, 128:136], 0.01, None, OP.mult)
                amx = spool.tile([P, NH], f32, tag="amx")
                nc.vector.tensor_tensor(amx[:], pa[:, 128:136], asb[:], OP.max)
                ea = spool.tile([P, NH], f32, tag="ea")
                nc.scalar.activation(ea[:], amx[:], AF.Exp)
                eae = spool.tile([P, NH, OUT_DIM], f32, tag="eae")
                nc.vector.tensor_copy(eae[:],
                                      ea[:, :, None].to_broadcast([P, NH, OUT_DIM]))
                pay = paypool.tile([P, 136], bf16, tag="pay")
                nc.vector.tensor_tensor(pay[:, 0:128], pa[:, 0:128],
                                        eae[:].rearrange("p a b -> p (a b)"), OP.mult)
                nc.scalar.copy(pay[:, 128:136], ea[:])
                for j in js:
                    pid = pair_of[(k, j)]
                    oh = ohpool.tile([P, P], bf16, tag="oh")
                    nc.sync.dma_start(oh[:], oneh_d[pid])
                    if first_k[j] == k:
                        acc[j] = accpsum.tile([P, 136], f32, tag="acc",
                                              name=f"acc{j}", space="PSUM")
                    nc.tensor.matmul(acc[j][:], oh[:], pay[:],
                                     start=(first_k[j] == k),
                                     stop=(last_k[j] == k),
                                     skip_group_check=True)
                for j in js:
                    if last_k[j] != k:
                        continue
                    aj = acc.pop(j)
                    sc = spool.tile([P, NH], f32, tag="sc")
                    nc.vector.tensor_scalar(sc[:], aj[:, 128:136], 1e-30, None,
                                            OP.max)
                    rc = spool.tile([P, NH], f32, tag="rc")
                    nc.vector.reciprocal(rc[:], sc[:])
                    rce = spool.tile([P, NH, OUT_DIM], f32, tag="rce")
                    nc.vector.tensor_copy(
                        rce[:], rc[:, :, None].to_broadcast([P, NH, OUT_DIM]))
                    wq = spool.tile([P, P], f32, tag="wq")
                    nc.vector.tensor_tensor(wq[:], aj[:, 0:128],
                                            rce[:].rearrange("p a b -> p (a b)"),
                                            OP.mult)
                    o16 = spool.tile([P, OUT_DIM], f32, tag="o16")
                    nc.vector.reduce_sum(
                        o16[:], wq[:].rearrange("p (h i) -> p i h", h=NH),
                        axis=mybir.AxisListType.X)
                    ob = spool.tile([P, OUT_DIM], f32, tag="ob")
                    nc.vector.tensor_tensor(ob[:], o16[:], bc_sb[:], OP.add)
                    nc.sync.dma_start(out_d[j * P:(j + 1) * P, :], ob[:])

    nc.compile()
    return nc


def kernel(**inputs):
    import os
    from concourse.bass_utils import run_bass_kernel_spmd

    num_nodes = int(inputs.pop("num_nodes", N_NODES))
    assert num_nodes == N_NODES
    plan, shared, percore = _host_prep(**inputs)
    nc = _build(plan)

    in_maps = []
    for c in range(N_CORES):
        m = dict(shared)
        m["gidx"] = np.ascontiguousarray(percore["gidx"][c])
        m["oneh"] = np.ascontiguousarray(percore["oneh"][c])
        in_maps.append(m)

    trace = bool(os.environ.get("KERNEL_TRACE"))
    res = run_bass_kernel_spmd(nc, in_maps, core_ids=list(range(N_CORES)),
                               trace=trace)
    global LAST_EXEC_NS, LAST_RESULTS
    LAST_EXEC_NS = getattr(res, "exec_time_ns", None)
    LAST_RESULTS = res

    full = np.empty((N_NODES, OUT_DIM), np.float32)
    for c in range(N_CORES):
        full[c * NPC:(c + 1) * NPC] = res.results[c]["out"][:NPC]
    # node chunks never flushed on device -> pure-bias rows
    for j in range(NODE_CHUNKS):
        if j not in plan["flushed"]:
            for c in range(N_CORES):
                lo = c * NPC + j * 128
                hi = min(c * NPC + min((j + 1) * 128, NPC), (c + 1) * NPC)
                if lo < hi:
                    full[lo:hi] = plan["bc_vec"][None, :]
    return full

